# revision 5
# baseline (speedup 1.0000x reference)
"""Multi-head cosine self-attention on 8 Trainium2 NeuronCores (Bass/Tile).

Problem: y = MHA(x) with L2-normalized q/k (cosine attention) and per-head
scaling sim / n**sigmoid(m);  x: [4, 2048, 1024], 16 heads of dim 64.

Sharding: core c handles batch c//2 and head-group c%2 (8 heads = 512 of the
1024 q/k/v features).  Each core computes its partial output
(attn_out_part @ Wo[rows]); the host sums the two partials per batch and adds
bo.  No collectives.

Per-core layout strategy (everything transposed, f' on partitions, bf16):
  - host passes x[b].T in bf16, so xT streams straight into SBUF
  - qT/kT = W.T @ xT via PE (f' on partitions, 2 heads per 128-partition
    tile); biases are added with a K=1 outer-product matmul into the same
    PSUM group, so evictions are plain copies with a free engine choice
  - row norms of q/k = matmul(ones_block, qT*qT) -> [2, n] per head pair;
    squares run on GpSimd (Pool) to keep DVE free; 1/(||q||*n^sig) =
    sqrt(cm_inv * reciprocal(norm^2)) via DVE reciprocal + one ACT Sqrt
  - the [2, n] factor row is broadcast to 128 partitions via a K=2 indicator
    matmul and applied in-place to qT/kT (DVE) so sim needs no further scaling
  - simT[j,i] = khatT.T @ qhatT per head with K=64 row-packing (2 heads
    concurrently in PE rows 0-63 / 64-127)
  - out2T[d,i] = sum_j v[j,d] * attnT[j,i] with M=64 col-packing (2 heads in
    PE cols 0-63 / 64-127 of one PSUM bank)
  - attn_outT kept in SBUF (bf16), final projection = attn_outT.T @ Wo_rows
Program order interleaves pair p's attention with pair p+1's q/k projections
and norm prep, and the final pair's attention with the output projection, so
the PE always has projection work to fill eviction-bound gaps.  sim PSUMs are
paired into 2-bank [128,1024] tiles; every PSUM eviction is assigned to DVE
or ACT by a greedy running-load balance (the two engines' eviction
throughput is the wall-clock floor).
"""

import os
import sys

for _p in ("/opt/trn_rl_repo",):
    if os.path.isdir(_p) and _p not in sys.path:
        sys.path.insert(0, _p)

from contextlib import ExitStack

import ml_dtypes
import numpy as np

import concourse.bacc as bacc
import concourse.mybir as mybir
import concourse.tile as tile
from concourse import bass_utils

P = 128
F = 1024  # model dim
H = 16  # total heads
HD = 64  # head dim
G = 2  # head groups (tensor-parallel factor)
FG = F // G  # 512 features per core
PAIRS = FG // P  # 4 head-pairs per core
KT = F // P  # 8 contraction tiles for the projections
NCORES = 8
F32 = mybir.dt.float32
BF = mybir.dt.bfloat16
AF = mybir.ActivationFunctionType


def _mm(nc, out, lhsT, rhs, **kw):
    nc.tensor.matmul(out, lhsT, rhs, **kw)


def build_core_program(nc, n=2048):
    NC = n // 512  # i-chunks
    NT = n // P  # n-tiles (= j-tiles)
    NTC = 512 // P  # n-tiles per i-chunk

    xt = nc.dram_tensor("xt", [P, NC, KT, 512], BF, kind="ExternalInput").ap()
    wq = nc.dram_tensor("wq", [P, PAIRS, KT, P], BF, kind="ExternalInput").ap()
    wk = nc.dram_tensor("wk", [P, PAIRS, KT, P], BF, kind="ExternalInput").ap()
    wv = nc.dram_tensor("wv", [P, KT, FG], BF, kind="ExternalInput").ap()
    wo = nc.dram_tensor("wo", [P, PAIRS, F], BF, kind="ExternalInput").ap()
    # bias rows: bqr/bkr[0, f'] for the 512 local features, bf16
    bqr = nc.dram_tensor("bqr", [1, FG], BF, kind="ExternalInput").ap()
    bkr = nc.dram_tensor("bkr", [1, FG], BF, kind="ExternalInput").ap()
    bvd = nc.dram_tensor("bv", [FG], BF, kind="ExternalInput").ap()
    # cmsq[a, p] = (n ** sigmoid(m))**-2 for local head 2p+a
    cmsq = nc.dram_tensor("cmsq", [2, PAIRS], F32, kind="ExternalInput").ap()
    cind = nc.dram_tensor("cind", [2, P], BF, kind="ExternalInput").ap()
    cblk = nc.dram_tensor("cblk", [P, 2], BF, kind="ExternalInput").ap()
    cones = nc.dram_tensor("cones", [1, 512], BF, kind="ExternalInput").ap()
    out = nc.dram_tensor("out", [n, F], F32, kind="ExternalOutput").ap()

    with tile.TileContext(nc) as tc, ExitStack() as ctx:
        const = ctx.enter_context(tc.tile_pool(name="const", bufs=1))
        persist = ctx.enter_context(tc.tile_pool(name="persist", bufs=1))
        work = ctx.enter_context(tc.tile_pool(name="work", bufs=1))
        ps = ctx.enter_context(tc.tile_pool(name="ps", bufs=1, space="PSUM"))

        # --- greedy DVE/ACT load balance for PSUM evictions --------------
        load = {"dve": 0.0, "act": 0.0}

        def copy_ps(dst, src, fd):
            dve_c = (120 + fd) / 0.96
            act_c = (172 + fd) / 1.2
            if load["dve"] + dve_c <= load["act"] + act_c:
                load["dve"] += dve_c
                nc.vector.tensor_copy(dst, src)
            else:
                load["act"] += act_c
                nc.scalar.copy(dst, src)

        # --- persistent activations & weights ----------------------------
        qT = persist.tile([P, PAIRS, n], BF)  # (x Wq + bq)^T, 2 heads/tile
        kT = persist.tile([P, PAIRS, n], BF)
        v = persist.tile([P, NT, FG], BF)  # x Wv + bv, natural layout
        aoT = persist.tile([P, PAIRS, n], BF)  # attn-out^T
        xall = persist.tile([P, NC, KT, 512], BF)
        wv_sb = persist.tile([P, KT, FG], BF)
        wo_sb = persist.tile([P, PAIRS, F], BF)

        def emit_qk_dma(pr):
            wfs = {}
            for wdr, wtag in ((wq, "wqf"), (wk, "wkf")):
                wf = work.tile([P, KT, P], BF, tag=wtag, bufs=2)
                nc.sync.dma_start(wf[:], wdr[:, pr])
                wfs[wtag] = wf
            return wfs

        # --- DMA order: first q/k weights + first x chunk, then the rest --
        wfs0 = emit_qk_dma(0)
        nc.sync.dma_start(xall[:, 0], xt[:, 0])
        nc.sync.dma_start(wv_sb[:], wv)
        for ic in range(1, NC):
            nc.sync.dma_start(xall[:, ic], xt[:, ic])
        nc.sync.dma_start(wo_sb[:], wo)

        # --- constants ---------------------------------------------------
        ones_blk = const.tile([P, 2], BF)  # block col-sums for head-pair norms
        nc.sync.dma_start(ones_blk[:], cblk)
        ind = const.tile([2, P], BF)  # partition-broadcast indicator
        nc.sync.dma_start(ind[:], cind)
        ones512 = const.tile([1, 512], BF)  # bias outer-product row
        nc.sync.dma_start(ones512[:], cones)
        zcol = const.tile([P, 1], F32)  # explicit zero bias for ACT
        nc.any.memset(zcol[:], 0.0)

        bq_row = const.tile([1, FG], BF)
        nc.sync.dma_start(bq_row[:], bqr)
        bk_row = const.tile([1, FG], BF)
        nc.sync.dma_start(bk_row[:], bkr)
        bv_sb = const.tile([1, FG], BF)
        nc.sync.dma_start(bv_sb[:], bvd[None, :])
        cm_sb = const.tile([2, PAIRS], F32)
        nc.sync.dma_start(cm_sb[:], cmsq)

        def emit_qk_chunk(pr, wfs, ic):
            # q and k projections of pair pr for one 512-row i-chunk;
            # bias added via K=1 outer product so eviction is a plain copy
            isl = slice(ic * 512, (ic + 1) * 512)
            for wtag, brow, dstT in (("wqf", bq_row, qT), ("wkf", bk_row, kT)):
                wf = wfs[wtag]
                pt = ps.tile([P, 512], F32, tag="mm", bufs=2)
                for k in range(KT):
                    _mm(nc, pt, wf[:, k, :], xall[:, ic, k, :],
                        start=(k == 0), stop=False)
                _mm(nc, pt, brow[:, pr * P:(pr + 1) * P], ones512,
                    start=False, stop=True)
                copy_ps(dstT[:, pr, isl], pt, 512)

        def emit_norm(pr):
            # 1/(||q|| * n^sig) and 1/||k|| as [2, n] bf16 rows, applied
            # in place to qT/kT so sim needs no further scaling.
            # rsqrt = sqrt(cm_inv * reciprocal(norm^2)), cm_inv = n^(-2*sig).
            for src, scale_ap in ((qT, cm_sb[:, pr:pr + 1]), (kT, None)):
                sq = work.tile([P, n], BF, tag="sq", bufs=2)
                nc.gpsimd.tensor_tensor(sq[:], src[:, pr, :], src[:, pr, :],
                                        mybir.AluOpType.mult)
                rowr = work.tile([2, n], BF, tag="rowr", bufs=2)
                for ch in range(NC):
                    csl = slice(ch * 512, (ch + 1) * 512)
                    nps = ps.tile([2, 512], F32, tag="mm", bufs=2)
                    _mm(nc, nps, ones_blk, sq[:, csl], start=True, stop=True)
                    row = work.tile([2, 512], F32, tag="row", bufs=2)
                    nc.vector.reciprocal(row[:], nps)
                    load["dve"] += 658
                    if scale_ap is not None:
                        nc.scalar.activation(rowr[:, csl], row, AF.Sqrt,
                                             bias=zcol[:2], scale=scale_ap)
                    else:
                        nc.scalar.activation(rowr[:, csl], row, AF.Sqrt,
                                             bias=zcol[:2])
                    load["act"] += 570
                # broadcast row across partitions and apply in place
                for ch in range(NC):
                    csl = slice(ch * 512, (ch + 1) * 512)
                    bps = ps.tile([P, 512], F32, tag="mm", bufs=2)
                    _mm(nc, bps, ind, rowr[:, csl], start=True, stop=True)
                    nc.vector.tensor_tensor(src[:, pr, csl], src[:, pr, csl],
                                            bps, mybir.AluOpType.mult)
                    load["dve"] += 690

        def emit_attn_chunk(pr, ic):
            # simT -> attnT -> out2T for one 512-col i-chunk of pair pr
            isl = slice(ic * 512, (ic + 1) * 512)
            avp = ps.tile([P, 512], F32, tag="av", bufs=2)
            for j in range(NT):
                jsl = slice(j * P, (j + 1) * P)
                sp2 = ps.tile([P, 1024], F32, tag="mm2", bufs=2)
                for po in (0, HD):  # head 2pr (rows 0-63), 2pr+1
                    _mm(nc, sp2[:, (po // HD) * 512:(po // HD) * 512 + 512],
                        kT[po:po + HD, pr, jsl],
                        qT[po:po + HD, pr, isl],
                        start=True, stop=True, tile_position=(po, 0))
                at = work.tile([P, 1024], BF, tag="at", bufs=6)
                copy_ps(at[:], sp2, 1024)
                for po in (0, HD):
                    _mm(nc, avp[po:po + HD, :],
                        v[:, j, pr * P + po:pr * P + po + HD],
                        at[:, (po // HD) * 512:(po // HD) * 512 + 512],
                        start=(j == 0), stop=(j == NT - 1),
                        tile_position=(0, po), skip_group_check=True)
            copy_ps(aoT[:, pr, isl], avp, 512)

        def emit_out_chunk(ic):
            # final projection for the NTC row-tiles of one i-chunk
            for t in range(NTC):
                nt = ic * NTC + t
                ntsl = slice(nt * P, (nt + 1) * P)
                ost = work.tile([P, F], F32, tag="ost", bufs=2)
                for fc in range(F // 512):
                    fsl = slice(fc * 512, (fc + 1) * 512)
                    pt2 = ps.tile([P, 512], F32, tag="mm", bufs=2)
                    for kt in range(PAIRS):
                        _mm(nc, pt2, aoT[:, kt, ntsl], wo_sb[:, kt, fsl],
                            start=(kt == 0), stop=(kt == PAIRS - 1))
                    copy_ps(ost[:, fsl], pt2, 512)
                nc.sync.dma_start(out[ntsl, :], ost[:])

        # ================= interleaved emission ==========================
        for ic in range(NC):
            emit_qk_chunk(0, wfs0, ic)
        emit_norm(0)

        # v projections: PE filler while pair 0's norm chain runs on DVE/ACT
        for ic in range(NC):
            for jt in range(NTC):
                nt_idx = ic * NTC + jt
                jsl = slice(jt * P, (jt + 1) * P)
                pt = ps.tile([P, FG], F32, tag="mm", bufs=2)
                for k in range(KT):
                    _mm(nc, pt, xall[:, ic, k, jsl], wv_sb[:, k, :],
                        start=(k == 0), stop=False)
                # + 1s^T bv outer product adds the bias to every row
                _mm(nc, pt, ones512[:, :P], bv_sb, start=False, stop=True)
                copy_ps(v[:, nt_idx, :], pt, 512)

        for pr in range(PAIRS):
            if pr < PAIRS - 1:
                wfs = emit_qk_dma(pr + 1)
            for ic in range(NC):
                emit_attn_chunk(pr, ic)
                if pr < PAIRS - 1:
                    emit_qk_chunk(pr + 1, wfs, ic)
                else:
                    emit_out_chunk(ic)
            if pr < PAIRS - 1:
                emit_norm(pr + 1)
    return nc


_CACHE = {}


def get_nc(n=2048):
    if n not in _CACHE:
        nc = bacc.Bacc("TRN2", target_bir_lowering=False, debug=False,
                       num_devices=NCORES)
        build_core_program(nc, n)
        nc.compile()
        _CACHE[n] = nc
    return _CACHE[n]


def _warr(W, sl):
    return np.ascontiguousarray(
        np.asarray(W, np.float32)[:, sl].reshape(KT, P, FG)
        .transpose(1, 0, 2)).astype(ml_dtypes.bfloat16)


def _warr_ft(W, sl):
    return np.ascontiguousarray(
        np.asarray(W, np.float32)[:, sl].reshape(KT, P, PAIRS, P)
        .transpose(1, 2, 0, 3)).astype(ml_dtypes.bfloat16)


_IND = np.zeros((2, P), ml_dtypes.bfloat16)
_IND[0, :HD] = 1.0
_IND[1, HD:] = 1.0
_BLK = np.zeros((P, 2), ml_dtypes.bfloat16)
_BLK[:HD, 0] = 1.0
_BLK[HD:, 1] = 1.0
_ONES = np.ones((1, 512), ml_dtypes.bfloat16)


def make_in_maps(x, Wq, bq, Wk, bk, Wv, bv, Wo, bo, m):
    n = x.shape[1]
    sig = 1.0 / (1.0 + np.exp(-np.asarray(m, np.float64)))
    scale = np.float64(n) ** sig  # [16] per-head n^sigmoid(m)
    in_maps = []
    for c in range(NCORES):
        bi, g = divmod(c, 2)
        sl = slice(g * FG, (g + 1) * FG)
        hsc = scale[g * (H // G):(g + 1) * (H // G)]  # 8 local heads
        cm = (hsc ** -2.0).reshape(PAIRS, 2).T  # [2, PAIRS], n^(-2*sig)
        xa = np.asarray(x[bi], np.float32)
        NCc = n // 512
        in_maps.append({
            "xt": np.ascontiguousarray(
                xa.reshape(NCc, 512, KT, P).transpose(3, 0, 2, 1))
                .astype(ml_dtypes.bfloat16),
            "wq": _warr_ft(Wq, sl), "wk": _warr_ft(Wk, sl), "wv": _warr(Wv, sl),
            "wo": np.ascontiguousarray(
                np.asarray(Wo, np.float32)[sl].reshape(PAIRS, P, F)
                .transpose(1, 0, 2).astype(ml_dtypes.bfloat16)),
            "bqr": np.asarray(bq, np.float32)[sl].reshape(1, FG).astype(ml_dtypes.bfloat16),
            "bkr": np.asarray(bk, np.float32)[sl].reshape(1, FG).astype(ml_dtypes.bfloat16),
            "bv": np.ascontiguousarray(np.asarray(bv, np.float32)[sl]).astype(ml_dtypes.bfloat16),
            "cmsq": np.ascontiguousarray(cm.astype(np.float32)),
            "cind": _IND,
            "cblk": _BLK,
            "cones": _ONES,
        })
    return in_maps


def kernel(x, Wq, bq, Wk, bk, Wv, bv, Wo, bo, m, _trace=False):
    x = np.asarray(x, np.float32)
    b, n, f = x.shape
    nc = get_nc(n)
    in_maps = make_in_maps(x, Wq, bq, Wk, bk, Wv, bv, Wo, bo, m)
    res = bass_utils.run_bass_kernel_spmd(nc, in_maps,
                                          core_ids=list(range(NCORES)),
                                          trace=_trace)
    outs = [r["out"] for r in res.results]
    y = np.empty((b, n, f), np.float32)
    for bi in range(b):
        y[bi] = outs[2 * bi] + outs[2 * bi + 1]
    y += np.asarray(bo, np.float32).reshape(1, 1, f)
    if _trace:
        kernel._last_results = res
    return y


if __name__ == "__main__":
    # build-only smoke test (no device)
    nc = bacc.Bacc("TRN2", target_bir_lowering=False, debug=False,
                   num_devices=NCORES)
    build_core_program(nc, n=int(sys.argv[1]) if len(sys.argv) > 1 else 2048)
    print("build OK")
